# revision 17
# baseline (speedup 1.0000x reference)
"""Trainium2 Bass kernel for nn_BoundaryPredictor2 (B=4, L=1500, D=512, NH=8).

Sharding: 8 cores = batch (4) x segment-half (2). Each core runs the full
boundary chain for its batch (duplicated within the pair) and pools its half
of the segments.

Key algebra vs the reference:
- hard = (soft > 0.5) == (p > 1-u) exactly (logit monotonicity), so the
  boundary decision needs no transcendentals; thr = 1-u precomputed on host.
- mlp(nrm(h)) is shared between the q (tokens :-1) and k (tokens 1:) branches.
- y = nrm(m + z) is never normalized: cos[l] = (qr[l]*kr[l+1])*rny[l]*rny[l+1].
- base[l,h] = hn[l]*veff[h]*HD^-0.5 with veff[h] = qh[h] @ Wpk[64h:64h+64,:],
  so keys are never materialized.
- hn itself is never materialized: x@(W*ln_g) for x=(h-mu)*rstd is computed as
  rstd*(h@Wg - mu*colsum(Wg)); the mu term is a rank-1 outer product matmul
  accumulated into the same PSUM group, rstd folds into the exp() activation
  scale (for base) and into a per-token e2 scale (for vals).
- GEMM precision: margins require |dcos| < 2.3e-4. fp32r rounds operands to
  11 mantissa bits; host splits W = Wh + Wl (round-to-nearest), kernel does
  2 matmul passes Wh@x + Wl@x with x fed as fp32r directly (HW rounds x).
  Residual error ~ 4.7e-5 in cos (simulated, >20x margin headroom).
- Segments are contiguous; pooling = (M^T @ (vals*e)) / (M^T @ e) with M the
  one-hot token->segment matrix built from a prefix scan of hard. The f-major
  pooling keeps 6 segment-chunk PSUM groups + 2 transposed-denominator groups
  live so the PE streams without per-mask vector round-trips.
"""
import numpy as np
from contextlib import ExitStack

import concourse.bass as bass
import concourse.bacc as bacc
import concourse.mybir as mybir
from concourse import tile

dt = mybir.dt
AF = mybir.ActivationFunctionType
ALU = mybir.AluOpType

B, L, D, NH, HD = 4, 1500, 512, 8, 64
EPS = 1e-8
PEPS = 1.1920929e-07
LT = 1536            # padded token count (12 tiles of 128)
NLT = LT // 128      # 12 l-tiles
LH = L // 2          # 750 (dma halves)
SH = 750             # segments per core (half of L)
SHP = 768            # padded (6 chunks of 128)
NSC = SHP // 128     # 6 s-chunks
KC = D // 128        # 4 contraction chunks
EXP_SHIFT = -4.0     # constant softmax shift (base observed in [-5.3, 5.6])

_nc_cache = {}


def _build(bias_f, debug=False):
    """Build the SPMD Bass program (same code for all cores; data differs)."""
    nc = bacc.Bacc("TRN2", target_bir_lowering=False, debug=False)

    def din(name, shape, dtype=dt.float32):
        return nc.dram_tensor(name, shape, dtype, kind="ExternalInput").ap()

    d_hT = din("hiddenT", (D, L), dt.float32r)
    d_thr = din("thr", (1, L))
    d_w = {n + s: din(n + s, (D, D), dt.float32r)
           for n in ("W1T", "W2T", "GT") for s in ("h", "l")}
    d_w["WpvT"] = din("WpvT", (D, D), dt.float32r)
    d_w["WpoT"] = din("WpoT", (D, D), dt.float32r)
    d_veff = din("veffT", (D, NH), dt.float32r)
    d_rn = din("rn_row", (1, LT))
    d_negmu = din("negmu_row", (1, LT), dt.float32r)
    d_rstdc = din("rstd_cols", (128, NLT))
    d_wbar_v = din("wbar_v", (1, D), dt.float32r)
    d_wbar_e = din("wbar_e", (1, NH), dt.float32r)
    d_iota = din("iota_s", (1, SHP))
    d_eye = din("eye", (128, 128))
    d_bias2 = din("bias2", (D, 2))
    d_out = nc.dram_tensor("out_half", (SH, D), dt.float32, kind="ExternalOutput").ap()
    dbg = {}
    if debug:
        for nm in ("cos_row", "hard_row", "seg_row", "rny_row"):
            dbg[nm] = nc.dram_tensor(nm, (1, LT), dt.float32, kind="ExternalOutput").ap()

        def dbg_dump(nm, ap):
            nc.sync.dma_start(dbg[nm][:], ap)
    else:
        def dbg_dump(nm, ap):
            pass

    with tile.TileContext(nc) as tc, ExitStack() as ctx:
        P = ctx.enter_context(tc.tile_pool(name="main", bufs=1))

        def fc(t, k, lo, n, w=LT):
            return t[:, k * w + lo:k * w + lo + n]

        def row(role, tag):
            return P.tile([1, LT], dt.float32, name=role, tag=f"row{tag}")

        def big(name, tag, cols=KC * LT, tdt=dt.float32):
            return P.tile([128, cols], tdt, name=name, tag=tag)

        # ---------- wave-1 DMAs ----------
        # sync: rn (critical for zT), then hidden^T halves, then small consts
        rn_row = row("rn_row", 4)
        nc.sync.dma_start(rn_row[:], d_rn[:])
        hT = P.tile([128, KC * LT], dt.float32r, name="hT", tag="A")

        def hf(k, lo, n):      # fp32 view of hT chunk
            return fc(hT, k, lo, n).bitcast(dt.float32)
        for k in range(KC):
            nc.sync.dma_start(fc(hT, k, 0, LH), d_hT[k * 128:(k + 1) * 128, 0:LH])
        for k in range(KC):
            nc.sync.dma_start(fc(hT, k, LH, L - LH), d_hT[k * 128:(k + 1) * 128, LH:L])

        # scalar: W1 pair, then Wpv + veff (pooling prep)
        def wtile(slot):
            return P.tile([128, KC * D], dt.float32r, name=slot, tag=slot)

        def wload(eng, t, name):
            eng.dma_start(t[:].rearrange("p (k d) -> p k d", k=KC),
                          d_w[name][:].rearrange("(k p) d -> p k d", k=KC))

        w1h, w1l = wtile("s0"), wtile("s1")
        w2h, w2l = wtile("s2"), wtile("s3")
        wpv = wtile("s4")
        wload(nc.scalar, w1h, "W1Th")
        wload(nc.scalar, w1l, "W1Tl")
        wload(nc.scalar, wpv, "WpvT")
        veff = P.tile([128, KC * NH], dt.float32r, name="veff_sb", tag="veff_sb")
        nc.scalar.dma_start(veff[:].rearrange("p (k h) -> p k h", k=KC),
                            d_veff[:].rearrange("(k p) h -> p k h", k=KC))

        # gpsimd: broadcast rn for zT, zero hT pads, then W2 pair
        bc1 = P.tile([128, LT], dt.float32, name="bc1", tag="bc1")
        nc.gpsimd.partition_broadcast(bc1[:], rn_row[:])
        for k in range(KC):   # zero pad columns (memset can't target fp32r)
            nc.gpsimd.tensor_scalar(fc(hT, k, L, LT - L), fc(hT, k, 0, LT - L),
                                    0.0, None, op0=ALU.mult)
        wload(nc.gpsimd, w2h, "W2Th")
        wload(nc.gpsimd, w2l, "W2Tl")

        # sync: remaining small constants
        eye = P.tile([128, 128], dt.float32, name="eye_sb", tag="eye_sb")
        nc.gpsimd.dma_start(eye[:], d_eye[:])
        bias2 = P.tile([128, KC * 2], dt.float32, name="bias2_sb", tag="bias2_sb")
        nc.scalar.dma_start(bias2[:].rearrange("p (k t) -> p k t", k=KC),
                            d_bias2[:].rearrange("(k p) t -> p k t", k=KC))

        def b1c(do):
            return bias2[:, do * 2:do * 2 + 1]

        def b2c(do):
            return bias2[:, do * 2 + 1:do * 2 + 2]

        iota_b = P.tile([128, SHP], dt.float32, name="iota_b", tag="iota_b")
        nc.gpsimd.dma_start(iota_b[:], d_iota[:].partition_broadcast(128))
        rstdc = P.tile([128, NLT], dt.float32, name="rstdc", tag="rstdc")
        nc.gpsimd.dma_start(rstdc[:], d_rstdc[:])
        wbar_v = P.tile([1, D], dt.float32r, name="wbar_v", tag="wbar_v")
        nc.gpsimd.dma_start(wbar_v[:], d_wbar_v[:])
        wbar_e = P.tile([1, NH], dt.float32r, name="wbar_e", tag="wbar_e")
        nc.gpsimd.dma_start(wbar_e[:], d_wbar_e[:])
        negmu = P.tile([1, LT], dt.float32r, name="negmu", tag="row6")
        nc.gpsimd.dma_start(negmu[:], d_negmu[:])
        thr_row = row("thr_row", 3)
        nc.gpsimd.dma_start(thr_row[:, 0:L], d_thr[:])

        ones_col = P.tile([128, 1], dt.float32, name="ones_col", tag="ones_col")
        nc.vector.memset(ones_col[:], 1.0)
        ones_r = P.tile([128, 1], dt.float32r, name="ones_r", tag="ones_r")
        nc.scalar.copy(ones_r[:], ones_col[:])
        eshift = P.tile([128, 1], dt.float32, name="eshift", tag="eshift")
        nc.vector.memset(eshift[:], EXP_SHIFT)

        # ============ z^T = h^T * rn ============
        zT = big("zT", "C", tdt=dt.float32r)
        for k in range(KC):
            nc.vector.tensor_tensor(fc(zT, k, 0, LH), hf(k, 0, LH),
                                    bc1[:, 0:LH], op=ALU.mult)
        for k in range(KC):
            nc.vector.tensor_tensor(fc(zT, k, LH, LT - LH), hf(k, LH, LT - LH),
                                    bc1[:, LH:LT], op=ALU.mult)

        # ============ 2-pass fp32r GEMM: out = (Wh + Wl) @ round11(x) ========
        def w_matmul(wh, wl, rhs, evac):
            with tc.tile_pool(name="ps_mm", bufs=4, space="PSUM") as PS:
                for lc in range(LT // 512):
                    for do in range(KC):
                        acc = PS.tile([128, 512], dt.float32, name="mmacc", tag="mmacc")
                        i = 0
                        for k in range(KC):
                            x_ap = fc(rhs, k, lc * 512, 512)
                            whk = wh[:, k * D + do * 128:k * D + (do + 1) * 128]
                            wlk = wl[:, k * D + do * 128:k * D + (do + 1) * 128]
                            nc.tensor.matmul(acc[:], whk, x_ap,
                                             start=(i == 0), stop=False)
                            i += 1
                            nc.tensor.matmul(acc[:], wlk, x_ap,
                                             start=False, stop=(i == 2 * KC - 1))
                            i += 1
                        evac(acc, do, lc)

        gT = big("gT", "B", tdt=dt.float32r)

        def evac_gelu(acc, do, lc):
            nc.scalar.activation(fc(gT, do, lc * 512, 512), acc[:], AF.Gelu,
                                 bias=b1c(do))

        w_matmul(w1h, w1l, zT, evac_gelu)

        yT = big("yT", "E", tdt=dt.float32r)
        ssy_row = row("ssy_row", 1)
        tmp_row = row("tmp_row", 5)

        # MLP2 with fused y^2 scratch + ssy reduction (pipelines into chain)
        with tc.tile_pool(name="ps_rowy", bufs=2, space="PSUM") as PSY, \
                tc.tile_pool(name="sqpool", bufs=5) as SQ:
            ssy_acc = {}

            def evac_y(acc, do, lc):
                nc.vector.scalar_tensor_tensor(
                    fc(yT, do, lc * 512, 512), acc[:], b2c(do),
                    fc(zT, do, lc * 512, 512).bitcast(dt.float32),
                    op0=ALU.add, op1=ALU.add)
                yv = fc(yT, do, lc * 512, 512).bitcast(dt.float32)
                sq = SQ.tile([128, 512], dt.float32r, name="sq", tag="sq")
                nc.gpsimd.tensor_tensor(sq[:], yv, yv, op=ALU.mult)
                if do == 0:
                    ssy_acc[lc] = PSY.tile([1, 512], dt.float32, name="racy", tag="racy")
                nc.tensor.matmul(ssy_acc[lc][:], ones_r[:], sq[:],
                                 start=(do == 0), stop=(do == KC - 1))
                if do == KC - 1:
                    nc.scalar.copy(ssy_row[:, lc * 512:(lc + 1) * 512], ssy_acc[lc][:])

            w_matmul(w2h, w2l, gT, evac_y)

        # G chain reuses w1 slots; wpo reuses w2h slot (sync queue, ordered).
        gqh, gql = w1h, w1l
        wload(nc.sync, gqh, "GTh")
        wload(nc.sync, gql, "GTl")
        wpo = w2h
        wload(nc.sync, wpo, "WpoT")

        # ============ rny (rows on vector while G chain runs on PE) =========
        rny_row = row("rny_row", 4)            # rn_row dead
        nc.scalar.activation(tmp_row[:], ssy_row[:], AF.Sqrt)
        nc.vector.tensor_scalar_max(tmp_row[:], tmp_row[:], EPS)
        nc.vector.reciprocal(rny_row[:], tmp_row[:])
        dbg_dump("rny_row", rny_row[:])
        rr_row = row("rr_row", 1)              # ssy_row dead; rr[l] = rny[l]*rny[l+1]
        nc.vector.memset(rr_row[:, L - 1:LT], 0.0)
        nc.vector.tensor_tensor(rr_row[:, 0:L - 1], rny_row[:, 0:L - 1],
                                rny_row[:, 1:L], op=ALU.mult)

        # ============ qr, kr, cos ============
        # gq = y @ G with G = Wq.T @ Wk; cos[l] = gq[l] . y[l+1]
        prodT = big("prodT", "C", tdt=dt.float32r)   # reuse zT slot (dead)

        def evac_gq(acc, do, lc):
            # prod[:, l] = gq[:, l] * y[:, l+1]; pad/tail zeroed after
            lo = lc * 512
            n = 512 if lo + 512 < L else (L - 1 - lo)
            nc.vector.tensor_tensor(fc(prodT, do, lo, n), acc[0:128, 0:n],
                                    fc(yT, do, lo + 1, n).bitcast(dt.float32),
                                    op=ALU.mult)
            if n < 512:
                nc.vector.tensor_scalar(fc(prodT, do, lo + n, LT - lo - n),
                                        acc[0:128, 0:LT - lo - n], 0.0, None,
                                        op0=ALU.mult)

        w_matmul(gqh, gql, yT, evac_gq)
        # cos = (ones @ prod) * rr, scaling fused into the psum evacuation
        cos_row = row("cos_row", 2)
        with tc.tile_pool(name="ps_rowc", bufs=2, space="PSUM") as PSR:
            for lc in range(LT // 512):
                acc = PSR.tile([1, 512], dt.float32, name="racc2", tag="racc2")
                for k in range(KC):
                    nc.tensor.matmul(acc[:], ones_r[:], fc(prodT, k, lc * 512, 512),
                                     start=(k == 0), stop=(k == KC - 1))
                nc.vector.tensor_tensor(cos_row[:, lc * 512:(lc + 1) * 512], acc[:],
                                        rr_row[:, lc * 512:(lc + 1) * 512], op=ALU.mult)
        dbg_dump("cos_row", cos_row[:])

        # ============ pooling-side tensors (PE filler during row math) ======
        e_t = P.tile([128, NLT * NH], dt.float32r, name="e_t", tag="e_t")
        e2_t = P.tile([128, NLT * NH], dt.float32, name="e2_t", tag="e2_t")
        base_all = P.tile([128, NLT * NH], dt.float32, name="base_all", tag="base_all")
        vals = big("vals", "E", cols=NLT * 512, tdt=dt.float32r)  # yT dead

        rstd3 = rstdc[:].unsqueeze(2).broadcast_to([128, NLT, NH])
        with tc.tile_pool(name="ps_pv", bufs=1, space="PSUM") as PS:
            bcc = PS.tile([128, NLT * NH], dt.float32, name="bcc", tag="bcc")
            for f in range(NLT):
                nmu = negmu[:, f * 128:(f + 1) * 128]
                bf = bcc[:, f * NH:(f + 1) * NH]
                nc.tensor.matmul(bf, nmu, wbar_e[:], start=True, stop=False)
                for k in range(KC):
                    nc.tensor.matmul(bf, fc(hT, k, f * 128, 128),
                                     veff[:, k * NH:(k + 1) * NH],
                                     start=False, stop=(k == KC - 1))
            # base = rstd * base_raw; e = exp(base + shift); e2 = e * rstd
            nc.vector.tensor_tensor(base_all[:].rearrange("p (f h) -> p f h", f=NLT),
                                    bcc[:].rearrange("p (f h) -> p f h", f=NLT),
                                    rstd3, op=ALU.mult)
            nc.scalar.activation(e_t[:], base_all[:], AF.Exp, bias=eshift[:])
            nc.vector.tensor_tensor(e2_t[:].rearrange("p (f h) -> p f h", f=NLT),
                                    e_t[:].bitcast(dt.float32).rearrange("p (f h) -> p f h", f=NLT),
                                    rstd3, op=ALU.mult)
            for f in range(NLT):
                nmu = negmu[:, f * 128:(f + 1) * 128]
                acc = PS.tile([128, 512], dt.float32, name="vacc", tag="vacc", bufs=4)
                nc.tensor.matmul(acc[:], nmu, wbar_v[:], start=True, stop=False)
                for k in range(KC):
                    nc.tensor.matmul(acc[:], fc(hT, k, f * 128, 128),
                                     wpv[:, k * D:(k + 1) * D],
                                     start=False, stop=(k == KC - 1))
                # X = vals_raw * e2, fused psum evacuation
                nc.vector.tensor_tensor(
                    fc(vals, f, 0, 512, w=512).rearrange("p (h j) -> p h j", h=NH),
                    acc[:].rearrange("p (h j) -> p h j", h=NH),
                    e2_t[:, f * NH:(f + 1) * NH].unsqueeze(2).broadcast_to([128, NH, HD]),
                    op=ALU.mult)

        # ============ boundary decision: hard = (p > thr) ============
        p_row = row("p_row", 1)
        nc.vector.tensor_scalar(p_row[:, 0:L - 1], cos_row[:, 0:L - 1], -0.5,
                                0.5 - 0.5 * bias_f, op0=ALU.mult, op1=ALU.add)
        nc.vector.memset(p_row[:, L - 1:LT], 0.0)
        nc.vector.tensor_scalar(p_row[:, 0:L], p_row[:, 0:L], PEPS, 1.0 - PEPS,
                                op0=ALU.max, op1=ALU.min)
        hard_row = row("hard_row", 4)          # rny_row dead
        nc.vector.memset(hard_row[:], 0.0)
        nc.vector.tensor_tensor(hard_row[:, 0:L], p_row[:, 0:L], thr_row[:, 0:L],
                                op=ALU.is_gt)
        hsum = P.tile([1, 1], dt.float32, name="hsum", tag="hsum")
        nc.vector.tensor_reduce(hsum[:], hard_row[:, 0:L], axis=mybir.AxisListType.X,
                                op=ALU.add)
        nc.vector.tensor_scalar(hsum[:], hsum[:], 0.0, None, op0=ALU.is_equal)
        nc.vector.tensor_tensor(hard_row[:, L - 1:L], hard_row[:, L - 1:L], hsum[:],
                                op=ALU.max)
        dbg_dump("hard_row", hard_row[:])

        # ============ seg = exclusive prefix sum; distribute to columns ======
        seg_row = row("seg_row", 2)            # cos_row dead
        nc.vector.tensor_tensor_scan(seg_row[:], hard_row[:], hard_row[:], 0.0,
                                     op0=ALU.add, op1=ALU.bypass)
        nc.vector.tensor_tensor(seg_row[:], seg_row[:], hard_row[:], op=ALU.subtract)
        nc.vector.memset(seg_row[:, L:LT], -1.0)
        dbg_dump("seg_row", seg_row[:])

        seg_cols = P.tile([128, NLT], dt.float32, name="seg_cols", tag="seg_cols")
        with tc.tile_pool(name="ps_segc", bufs=1, space="PSUM") as PSC:
            pcol = PSC.tile([128, NLT], dt.float32, name="pcol", tag="pcol")
            for f in range(NLT):
                nc.tensor.matmul(pcol[:, f:f + 1], seg_row[0:1, f * 128:(f + 1) * 128],
                                 ones_col[0:1, 0:1], start=True, stop=True)
            nc.vector.tensor_copy(seg_cols[:], pcol[:])

        # ============ segment pooling: two halves of 3 segment-chunks ======
        # Per half: 3 live accx PSUM groups + 1 transposed-denominator group,
        # then rinv + pooled evac + transposes + output GEMM for those chunks.
        pooled = big("pooled", "B", cols=NSC * 512)     # gT slot dead
        pooledT = big("pooledT", "A", cols=KC * SHP, tdt=dt.float32r)  # hT dead after vacc
        MS = ctx.enter_context(tc.tile_pool(name="mscr", bufs=3))
        with tc.tile_pool(name="ps_seg", bufs=1, space="PSUM") as PS, \
                tc.tile_pool(name="ps_fin", bufs=1, space="PSUM") as PF:
            for half in range(2):
                scs = [half * 3, half * 3 + 1, half * 3 + 2]
                lo = half * 384
                accx = [PS.tile([128, 512], dt.float32, name=f"accx{j}",
                                tag=f"accx{j}", bufs=1) for j in range(3)]
                accd = PS.tile([8, 512], dt.float32, name="accd", tag="accd", bufs=1)
                for f in range(NLT):
                    m_f = MS.tile([128, 384], dt.float32r, name="m_f", tag="m_f")
                    nc.vector.tensor_scalar(m_f[:], iota_b[:, lo:lo + 384],
                                            seg_cols[:, f:f + 1], None, op0=ALU.is_equal)
                    st, sp = (f == 0), (f == NLT - 1)
                    for j in range(3):
                        nc.tensor.matmul(accx[j][:], m_f[:, j * 128:(j + 1) * 128],
                                         fc(vals, f, 0, 512, w=512), start=st, stop=sp)
                    nc.tensor.matmul(accd[:, 0:384], e_t[:, f * NH:(f + 1) * NH],
                                     m_f[:], start=st, stop=sp)
                # denom -> SBUF -> transpose -> rinv in [seg, head] layout
                denT = P.tile([8, 384], dt.float32, name="denT", tag="denT")
                nc.vector.tensor_copy(denT[:], accd[:, 0:384])
                dtr = PF.tile([128, 3 * NH], dt.float32, name="dtr", tag="dtr", bufs=1)
                for j in range(3):
                    nc.tensor.transpose(dtr[:, j * NH:(j + 1) * NH],
                                        denT[:, j * 128:(j + 1) * 128],
                                        eye[0:NH, 0:NH])
                msk = P.tile([128, 3 * NH], dt.float32, name="msk", tag="msk")
                rinvc = P.tile([128, 3 * NH], dt.float32, name="rinvc", tag="rinvc")
                nc.vector.tensor_scalar(msk[:], dtr[:], 0.0, None, op0=ALU.is_gt)
                nc.vector.tensor_scalar(rinvc[:], msk[:], -1.0, 1.0,
                                        op0=ALU.mult, op1=ALU.add)      # 1-mask
                nc.vector.tensor_tensor(rinvc[:], rinvc[:], dtr[:], op=ALU.add)
                nc.vector.reciprocal(rinvc[:], rinvc[:])
                nc.vector.tensor_tensor(rinvc[:], rinvc[:], msk[:], op=ALU.mult)
                for j, sc in enumerate(scs):
                    nc.vector.tensor_tensor(
                        pooled[:, sc * 512:(sc + 1) * 512].rearrange("p (h j) -> p h j", h=NH),
                        accx[j][:].rearrange("p (h j) -> p h j", h=NH),
                        rinvc[:, j * NH:(j + 1) * NH].unsqueeze(2).broadcast_to([128, NH, HD]),
                        op=ALU.mult)
                # transpose + output GEMM for this half's chunks
                for sc in scs:
                    ptr = PF.tile([128, 512], dt.float32, name="ptr", tag="ptr", bufs=2)
                    for ch in range(KC):
                        nc.tensor.transpose(
                            ptr[:, ch * 128:(ch + 1) * 128],
                            pooled[:, sc * 512 + ch * 128:sc * 512 + (ch + 1) * 128],
                            eye[:])
                    nc.vector.tensor_copy(
                        pooledT[:].rearrange("p (c s) -> p c s", c=KC)[:, :, sc * 128:(sc + 1) * 128],
                        ptr[:].rearrange("p (c l) -> p c l", c=KC))
                for sc in scs:
                    nrows = min(128, SH - sc * 128)
                    if nrows <= 0:
                        break
                    acco = PF.tile([128, D], dt.float32, name="acco", tag="acco", bufs=1)
                    for ch in range(KC):
                        nc.tensor.matmul(
                            acco[:], pooledT[:, ch * SHP + sc * 128:ch * SHP + (sc + 1) * 128],
                            wpo[:, ch * D:(ch + 1) * D],
                            start=(ch == 0), stop=(ch == KC - 1))
                    o_sb = pooled[:, sc * 512:sc * 512 + D].bitcast(dt.float32)
                    nc.vector.tensor_copy(o_sb, acco[:])
                    nc.sync.dma_start(d_out[sc * 128:sc * 128 + nrows, :], o_sb[0:nrows, :])

    nc.compile()
    return nc


def _rnd11(x):
    """Round fp32 to 11 explicit mantissa bits (fp32r operand precision)."""
    u = np.ascontiguousarray(x).astype(np.float32).view(np.uint32)
    out = ((u + np.uint32(1 << 11)) & np.uint32(0xFFFFF000)).view(np.float32)
    return out


def _prep_host(inputs):
    """Host-side prep: transposes, veff fold, hi/lo weight splits, per-core in_maps."""
    f32 = np.float32
    hidden = np.asarray(inputs["hidden"], f32)
    u_noise = np.asarray(inputs["u_noise"], f32)
    W1 = np.asarray(inputs["W1"], f32)
    W2 = np.asarray(inputs["W2"], f32)
    Wq = np.asarray(inputs["Wq"], f32)
    Wk = np.asarray(inputs["Wk"], f32)
    Wpk = np.asarray(inputs["Wpk"], f32)
    Wpv = np.asarray(inputs["Wpv"], f32)
    Wpo = np.asarray(inputs["Wpo"], f32)
    lq = np.asarray(inputs["learned_query"], f32)
    ln_g = np.asarray(inputs["ln_g"], f32)
    ln_b = np.asarray(inputs["ln_b"], f32)
    b1 = np.asarray(inputs["b1"], f32)
    b2 = np.asarray(inputs["b2"], f32)
    lengths = np.asarray(inputs["lengths"], f32)
    bias_f = float(np.asarray(inputs["sim_bias"], f32))
    assert np.all(lengths == 1.0), "kernel specialized for lengths == 1"
    assert np.all(ln_b == 0.0), "kernel assumes ln_b == 0 (fold not implemented)"

    Wpv_f = Wpv * ln_g[None, :]
    Wpk_f = Wpk * ln_g[None, :]
    qh = lq.reshape(NH, HD)
    veff = np.einsum("hj,hji->hi", qh, Wpk_f.reshape(NH, HD, D)) * f32(HD ** -0.5)
    veffT = np.ascontiguousarray(veff.T)
    WpvT = np.ascontiguousarray(Wpv_f.T)

    common = {
        "WpvT": WpvT, "WpoT": np.ascontiguousarray(Wpo.T),
        "veffT": veffT, "eye": np.eye(128, dtype=f32),
        "bias2": np.ascontiguousarray(np.stack([b1, b2], 1)),
        "wbar_v": WpvT.sum(0, dtype=np.float64).astype(f32).reshape(1, D),
        "wbar_e": veffT.sum(0, dtype=np.float64).astype(f32).reshape(1, NH),
    }
    G = (Wq.T.astype(np.float64) @ Wk.astype(np.float64)).astype(f32)
    for nm, w in (("W1T", W1), ("W2T", W2), ("GT", G.T)):
        wt = np.ascontiguousarray(w.T)
        hi = _rnd11(wt)
        common[nm + "h"] = hi
        common[nm + "l"] = np.ascontiguousarray(wt - hi)

    # per-batch token stats on host (pure input preprocessing)
    ssq = np.einsum("bld,bld->bl", hidden, hidden, dtype=np.float64)
    rn = (1.0 / np.maximum(np.sqrt(ssq), EPS)).astype(f32)
    mu = hidden.mean(-1, dtype=np.float64).astype(f32)
    var = (ssq / D - mu.astype(np.float64) ** 2)
    rstd = (1.0 / np.sqrt(var + 1e-5)).astype(f32)
    thr = (1.0 - np.clip(u_noise, PEPS, 1.0 - PEPS)).astype(f32)

    in_maps = []
    for c in range(8):
        b, sh = divmod(c, 2)
        m = dict(common)
        m["hiddenT"] = np.ascontiguousarray(hidden[b].T)
        m["thr"] = np.ascontiguousarray(thr[b].reshape(1, L))
        rn_r = np.zeros((1, LT), f32); rn_r[0, :L] = rn[b]
        m["rn_row"] = rn_r
        nm_r = np.zeros((1, LT), f32); nm_r[0, :L] = -mu[b]
        m["negmu_row"] = nm_r
        rs = np.zeros(128 * NLT, f32)
        rs[:L] = rstd[b]
        m["rstd_cols"] = np.ascontiguousarray(rs.reshape(NLT, 128).T)
        m["iota_s"] = (2.0 * np.arange(SHP, dtype=f32) + sh).reshape(1, SHP)
        in_maps.append(m)
    return in_maps, bias_f


def get_nc(bias_f, debug=False):
    key = (round(bias_f, 9), debug)
    if key not in _nc_cache:
        _nc_cache[key] = _build(bias_f, debug=debug)
    return _nc_cache[key]


def kernel(**inputs):
    from concourse.bass_utils import run_bass_kernel_spmd
    in_maps, bias_f = _prep_host(inputs)
    nc = get_nc(bias_f)
    res = run_bass_kernel_spmd(nc, in_maps, list(range(8))).results
    out = np.zeros((B, L, D), np.float32)
    for c in range(8):
        b, sh = divmod(c, 2)
        out[b, sh:sh + 2 * SH:2, :] = res[c]["out_half"]
    return out


# revision 18
# speedup vs baseline: 1.1754x; 1.1754x over previous
"""Trainium2 Bass kernel for nn_BoundaryPredictor2 (B=4, L=1500, D=512, NH=8).

Sharding: 8 cores = batch (4) x segment-half (2). Each core runs the full
boundary chain for its batch (duplicated within the pair) and pools its half
of the segments.

Key algebra vs the reference:
- hard = (soft > 0.5) == (p > 1-u) exactly (logit monotonicity), so the
  boundary decision needs no transcendentals; thr = 1-u precomputed on host.
- mlp(nrm(h)) is shared between the q (tokens :-1) and k (tokens 1:) branches.
- y = nrm(m + z) is never normalized: cos[l] = (qr[l]*kr[l+1])*rny[l]*rny[l+1].
- base[l,h] = hn[l]*veff[h]*HD^-0.5 with veff[h] = qh[h] @ Wpk[64h:64h+64,:],
  so keys are never materialized.
- hn itself is never materialized: x@(W*ln_g) for x=(h-mu)*rstd is computed as
  rstd*(h@Wg - mu*colsum(Wg)); the mu term is a rank-1 outer product matmul
  accumulated into the same PSUM group, rstd folds into a batched base scale
  (for e) and a per-token e2 scale (for vals).
- GEMM precision: margins require |dcos| < 2.3e-4. fp32r rounds operands to
  11 mantissa bits; host splits W = Wh + Wl (round-to-nearest), kernel does
  2 matmul passes Wh@x + Wl@x with x fed as fp32r directly (HW rounds x).
  Residual error ~ 4.7e-5 in cos (simulated, >20x margin headroom).
- Segments are contiguous; pooling = (M^T @ (vals*e)) / (M^T @ e) with M the
  one-hot token->segment matrix built from a prefix scan of hard; f-major
  with 6 live accx groups + 2 transposed-denominator groups so the PE streams.
- All device tiles are DMA'd as single fat 2D copies from host-prearranged
  buffers (SBUF layout precomputed in numpy) to minimize issue count and
  maximize DMA run length.
"""
import numpy as np
from contextlib import ExitStack

import concourse.bass as bass
import concourse.bacc as bacc
import concourse.mybir as mybir
from concourse import tile

dt = mybir.dt
AF = mybir.ActivationFunctionType
ALU = mybir.AluOpType

B, L, D, NH, HD = 4, 1500, 512, 8, 64
EPS = 1e-8
PEPS = 1.1920929e-07
LT = 1536            # padded token count (12 tiles of 128)
NLT = LT // 128      # 12 l-tiles
SH = 750             # segments per core (half of L)
SHP = 768            # padded (6 chunks of 128)
NSC = SHP // 128     # 6 s-chunks
KC = D // 128        # 4 contraction chunks
EXP_SHIFT = -4.0     # constant softmax shift (base observed in [-5.3, 5.6])

_nc_cache = {}


def _build(bias_f, debug=False):
    """Build the SPMD Bass program (same code for all cores; data differs)."""
    nc = bacc.Bacc("TRN2", target_bir_lowering=False, debug=False)

    def din(name, shape, dtype=dt.float32):
        return nc.dram_tensor(name, shape, dtype, kind="ExternalInput").ap()

    # all inputs are host-prearranged to the exact SBUF tile layout
    d_hT = din("hT_dev", (128, KC * LT), dt.float32r)
    d_w = {n + s: din(n + s, (128, KC * D), dt.float32r)
           for n in ("W1T", "W2T", "GT") for s in ("h", "l")}
    d_w["WpvT"] = din("WpvT", (128, KC * D), dt.float32r)
    d_w["WpoT"] = din("WpoT", (128, KC * D), dt.float32r)
    d_veff = din("veffT", (128, KC * NH), dt.float32r)
    d_bias2 = din("bias2", (128, KC * 2))
    d_rn = din("rn_row", (1, LT))
    d_negmu = din("negmu_row", (1, LT), dt.float32r)
    d_rstdc = din("rstd_cols", (128, NLT))
    d_wbar_v = din("wbar_v", (1, D), dt.float32r)
    d_wbar_e = din("wbar_e", (1, NH), dt.float32r)
    d_iota = din("iota_s", (1, SHP))
    d_eye = din("eye", (128, 128))
    d_thr = din("thr", (1, L))
    d_out = nc.dram_tensor("out_half", (SH, D), dt.float32, kind="ExternalOutput").ap()
    dbg = {}
    if debug:
        for nm in ("cos_row", "hard_row", "seg_row", "rny_row"):
            dbg[nm] = nc.dram_tensor(nm, (1, LT), dt.float32, kind="ExternalOutput").ap()

        def dbg_dump(nm, ap):
            nc.sync.dma_start(dbg[nm][:], ap)
    else:
        def dbg_dump(nm, ap):
            pass

    with tile.TileContext(nc) as tc, ExitStack() as ctx:
        P = ctx.enter_context(tc.tile_pool(name="main", bufs=1))

        def fc(t, k, lo, n, w=LT):
            return t[:, k * w + lo:k * w + lo + n]

        def row(role, tag):
            return P.tile([1, LT], dt.float32, name=role, tag=f"row{tag}")

        def big(name, tag, cols=KC * LT, tdt=dt.float32):
            return P.tile([128, cols], tdt, name=name, tag=tag)

        # ---------- wave-1 DMAs: one fat descriptor per tensor ----------
        # sync: hidden^T (the critical 3MB), later G/Wpo second wave + outputs
        hT = P.tile([128, KC * LT], dt.float32r, name="hT", tag="A")
        nc.sync.dma_start(hT[:], d_hT[:])

        def hf(k, lo, n):      # fp32 view of hT chunk
            return fc(hT, k, lo, n).bitcast(dt.float32)

        # scalar: W1 pair, Wpv, veff, biases
        def wtile(slot):
            return P.tile([128, KC * D], dt.float32r, name=slot, tag=slot)

        w1h, w1l = wtile("s0"), wtile("s1")
        w2h, w2l = wtile("s2"), wtile("s3")
        wpv = wtile("s4")
        nc.scalar.dma_start(w1h[:], d_w["W1Th"][:])
        nc.scalar.dma_start(w1l[:], d_w["W1Tl"][:])
        nc.scalar.dma_start(wpv[:], d_w["WpvT"][:])
        veff = P.tile([128, KC * NH], dt.float32r, name="veff_sb", tag="veff_sb")
        nc.scalar.dma_start(veff[:], d_veff[:])
        bias2 = P.tile([128, KC * 2], dt.float32, name="bias2_sb", tag="bias2_sb")
        nc.scalar.dma_start(bias2[:], d_bias2[:])

        def b1c(do):
            return bias2[:, do * 2:do * 2 + 1]

        def b2c(do):
            return bias2[:, do * 2 + 1:do * 2 + 2]

        # gpsimd: rn first (feeds bc1), W2 pair, then small consts
        rn_row = row("rn_row", 4)
        nc.gpsimd.dma_start(rn_row[:], d_rn[:])
        bc1 = P.tile([128, LT], dt.float32, name="bc1", tag="bc1")
        nc.gpsimd.partition_broadcast(bc1[:], rn_row[:])
        nc.gpsimd.dma_start(w2h[:], d_w["W2Th"][:])
        nc.gpsimd.dma_start(w2l[:], d_w["W2Tl"][:])
        eye = P.tile([128, 128], dt.float32, name="eye_sb", tag="eye_sb")
        nc.gpsimd.dma_start(eye[:], d_eye[:])
        iota_b = P.tile([128, SHP], dt.float32, name="iota_b", tag="iota_b")
        nc.gpsimd.dma_start(iota_b[:], d_iota[:].partition_broadcast(128))
        rstdc = P.tile([128, NLT], dt.float32, name="rstdc", tag="rstdc")
        nc.gpsimd.dma_start(rstdc[:], d_rstdc[:])
        wbar_v = P.tile([1, D], dt.float32r, name="wbar_v", tag="wbar_v")
        nc.gpsimd.dma_start(wbar_v[:], d_wbar_v[:])
        wbar_e = P.tile([1, NH], dt.float32r, name="wbar_e", tag="wbar_e")
        nc.gpsimd.dma_start(wbar_e[:], d_wbar_e[:])
        negmu = P.tile([1, LT], dt.float32r, name="negmu", tag="row6")
        nc.gpsimd.dma_start(negmu[:], d_negmu[:])
        thr_row = row("thr_row", 3)
        nc.gpsimd.dma_start(thr_row[:, 0:L], d_thr[:])

        ones_col = P.tile([128, 1], dt.float32, name="ones_col", tag="ones_col")
        nc.vector.memset(ones_col[:], 1.0)
        ones_r = P.tile([128, 1], dt.float32r, name="ones_r", tag="ones_r")
        nc.scalar.copy(ones_r[:], ones_col[:])
        eshift = P.tile([128, 1], dt.float32, name="eshift", tag="eshift")
        nc.vector.memset(eshift[:], EXP_SHIFT)

        # ============ z^T = h^T * rn ============
        zT = big("zT", "C", tdt=dt.float32r)
        for k in range(KC):
            nc.vector.tensor_tensor(fc(zT, k, 0, LT), hf(k, 0, LT),
                                    bc1[:], op=ALU.mult)

        # ============ 2-pass fp32r GEMM: out = (Wh + Wl) @ round11(x) ========
        def w_matmul(wh, wl, rhs, evac):
            with tc.tile_pool(name="ps_mm", bufs=4, space="PSUM") as PS:
                for lc in range(LT // 512):
                    for do in range(KC):
                        acc = PS.tile([128, 512], dt.float32, name="mmacc", tag="mmacc")
                        i = 0
                        for k in range(KC):
                            x_ap = fc(rhs, k, lc * 512, 512)
                            whk = wh[:, k * D + do * 128:k * D + (do + 1) * 128]
                            wlk = wl[:, k * D + do * 128:k * D + (do + 1) * 128]
                            nc.tensor.matmul(acc[:], whk, x_ap,
                                             start=(i == 0), stop=False)
                            i += 1
                            nc.tensor.matmul(acc[:], wlk, x_ap,
                                             start=False, stop=(i == 2 * KC - 1))
                            i += 1
                        evac(acc, do, lc)

        gT = big("gT", "B", tdt=dt.float32r)

        def evac_gelu(acc, do, lc):
            nc.scalar.activation(fc(gT, do, lc * 512, 512), acc[:], AF.Gelu,
                                 bias=b1c(do))

        w_matmul(w1h, w1l, zT, evac_gelu)

        yT = big("yT", "E", tdt=dt.float32r)
        ssy_row = row("ssy_row", 1)
        tmp_row = row("tmp_row", 5)

        # MLP2 with fused y^2 scratch + ssy reduction (pipelines into chain)
        with tc.tile_pool(name="ps_rowy", bufs=2, space="PSUM") as PSY, \
                tc.tile_pool(name="sqpool", bufs=5) as SQ:
            ssy_acc = {}

            def evac_y(acc, do, lc):
                nc.vector.scalar_tensor_tensor(
                    fc(yT, do, lc * 512, 512), acc[:], b2c(do),
                    fc(zT, do, lc * 512, 512).bitcast(dt.float32),
                    op0=ALU.add, op1=ALU.add)
                yv = fc(yT, do, lc * 512, 512).bitcast(dt.float32)
                sq = SQ.tile([128, 512], dt.float32r, name="sq", tag="sq")
                nc.gpsimd.tensor_tensor(sq[:], yv, yv, op=ALU.mult)
                if do == 0:
                    ssy_acc[lc] = PSY.tile([1, 512], dt.float32, name="racy", tag="racy")
                nc.tensor.matmul(ssy_acc[lc][:], ones_r[:], sq[:],
                                 start=(do == 0), stop=(do == KC - 1))
                if do == KC - 1:
                    nc.scalar.copy(ssy_row[:, lc * 512:(lc + 1) * 512], ssy_acc[lc][:])

            w_matmul(w2h, w2l, gT, evac_y)

        # G chain reuses w1 slots; wpo reuses w2h slot (sync queue, ordered).
        gqh, gql = w1h, w1l
        nc.sync.dma_start(gqh[:], d_w["GTh"][:])
        nc.sync.dma_start(gql[:], d_w["GTl"][:])
        wpo = w2h
        nc.sync.dma_start(wpo[:], d_w["WpoT"][:])

        # ============ rny (rows on vector while G chain runs on PE) =========
        rny_row = row("rny_row", 4)            # rn_row dead
        nc.scalar.activation(tmp_row[:], ssy_row[:], AF.Sqrt)
        nc.vector.tensor_scalar_max(tmp_row[:], tmp_row[:], EPS)
        nc.vector.reciprocal(rny_row[:], tmp_row[:])
        dbg_dump("rny_row", rny_row[:])
        rr_row = row("rr_row", 1)              # ssy_row dead; rr[l] = rny[l]*rny[l+1]
        nc.vector.memset(rr_row[:, L - 1:LT], 0.0)
        nc.vector.tensor_tensor(rr_row[:, 0:L - 1], rny_row[:, 0:L - 1],
                                rny_row[:, 1:L], op=ALU.mult)

        # ============ qr, kr, cos ============
        # gq = y @ G with G = Wq.T @ Wk; cos[l] = gq[l] . y[l+1]
        prodT = big("prodT", "C", tdt=dt.float32r)   # reuse zT slot (dead)

        def evac_gq(acc, do, lc):
            # prod[:, l] = gq[:, l] * y[:, l+1]; pad/tail zeroed after
            lo = lc * 512
            n = 512 if lo + 512 < L else (L - 1 - lo)
            nc.vector.tensor_tensor(fc(prodT, do, lo, n), acc[0:128, 0:n],
                                    fc(yT, do, lo + 1, n).bitcast(dt.float32),
                                    op=ALU.mult)
            if n < 512:
                nc.vector.tensor_scalar(fc(prodT, do, lo + n, LT - lo - n),
                                        acc[0:128, 0:LT - lo - n], 0.0, None,
                                        op0=ALU.mult)

        w_matmul(gqh, gql, yT, evac_gq)
        # cos = (ones @ prod) * rr, scaling fused into the psum evacuation
        cos_row = row("cos_row", 2)
        with tc.tile_pool(name="ps_rowc", bufs=2, space="PSUM") as PSR:
            for lc in range(LT // 512):
                acc = PSR.tile([1, 512], dt.float32, name="racc2", tag="racc2")
                for k in range(KC):
                    nc.tensor.matmul(acc[:], ones_r[:], fc(prodT, k, lc * 512, 512),
                                     start=(k == 0), stop=(k == KC - 1))
                nc.vector.tensor_tensor(cos_row[:, lc * 512:(lc + 1) * 512], acc[:],
                                        rr_row[:, lc * 512:(lc + 1) * 512], op=ALU.mult)
        dbg_dump("cos_row", cos_row[:])

        # ============ pooling-side tensors (PE filler during row math) ======
        e_t = P.tile([128, NLT * NH], dt.float32r, name="e_t", tag="e_t")
        e2_t = P.tile([128, NLT * NH], dt.float32, name="e2_t", tag="e2_t")
        base_all = P.tile([128, NLT * NH], dt.float32, name="base_all", tag="base_all")
        vals = big("vals", "E", cols=NLT * 512, tdt=dt.float32r)  # yT dead

        rstd3 = rstdc[:].unsqueeze(2).broadcast_to([128, NLT, NH])
        with tc.tile_pool(name="ps_pv", bufs=1, space="PSUM") as PS:
            bcc = PS.tile([128, NLT * NH], dt.float32, name="bcc", tag="bcc")
            for f in range(NLT):
                nmu = negmu[:, f * 128:(f + 1) * 128]
                bf = bcc[:, f * NH:(f + 1) * NH]
                nc.tensor.matmul(bf, nmu, wbar_e[:], start=True, stop=False)
                for k in range(KC):
                    nc.tensor.matmul(bf, fc(hT, k, f * 128, 128),
                                     veff[:, k * NH:(k + 1) * NH],
                                     start=False, stop=(k == KC - 1))
            # base = rstd * base_raw; e = exp(base + shift); e2 = e * rstd
            nc.vector.tensor_tensor(base_all[:].rearrange("p (f h) -> p f h", f=NLT),
                                    bcc[:].rearrange("p (f h) -> p f h", f=NLT),
                                    rstd3, op=ALU.mult)
            nc.scalar.activation(e_t[:], base_all[:], AF.Exp, bias=eshift[:])
            nc.vector.tensor_tensor(e2_t[:].rearrange("p (f h) -> p f h", f=NLT),
                                    e_t[:].bitcast(dt.float32).rearrange("p (f h) -> p f h", f=NLT),
                                    rstd3, op=ALU.mult)
            for f in range(NLT):
                nmu = negmu[:, f * 128:(f + 1) * 128]
                acc = PS.tile([128, 512], dt.float32, name="vacc", tag="vacc", bufs=4)
                nc.tensor.matmul(acc[:], nmu, wbar_v[:], start=True, stop=False)
                for k in range(KC):
                    nc.tensor.matmul(acc[:], fc(hT, k, f * 128, 128),
                                     wpv[:, k * D:(k + 1) * D],
                                     start=False, stop=(k == KC - 1))
                # X = vals_raw * e2, fused psum evacuation
                nc.vector.tensor_tensor(
                    fc(vals, f, 0, 512, w=512).rearrange("p (h j) -> p h j", h=NH),
                    acc[:].rearrange("p (h j) -> p h j", h=NH),
                    e2_t[:, f * NH:(f + 1) * NH].unsqueeze(2).broadcast_to([128, NH, HD]),
                    op=ALU.mult)

        # ============ boundary decision: hard = (p > thr) ============
        p_row = row("p_row", 1)
        nc.vector.tensor_scalar(p_row[:, 0:L - 1], cos_row[:, 0:L - 1], -0.5,
                                0.5 - 0.5 * bias_f, op0=ALU.mult, op1=ALU.add)
        nc.vector.memset(p_row[:, L - 1:LT], 0.0)
        nc.vector.tensor_scalar(p_row[:, 0:L], p_row[:, 0:L], PEPS, 1.0 - PEPS,
                                op0=ALU.max, op1=ALU.min)
        hard_row = row("hard_row", 4)          # rny_row dead
        nc.vector.memset(hard_row[:], 0.0)
        nc.vector.tensor_tensor(hard_row[:, 0:L], p_row[:, 0:L], thr_row[:, 0:L],
                                op=ALU.is_gt)
        hsum = P.tile([1, 1], dt.float32, name="hsum", tag="hsum")
        nc.vector.tensor_reduce(hsum[:], hard_row[:, 0:L], axis=mybir.AxisListType.X,
                                op=ALU.add)
        nc.vector.tensor_scalar(hsum[:], hsum[:], 0.0, None, op0=ALU.is_equal)
        nc.vector.tensor_tensor(hard_row[:, L - 1:L], hard_row[:, L - 1:L], hsum[:],
                                op=ALU.max)
        dbg_dump("hard_row", hard_row[:])

        # ============ seg = exclusive prefix sum; distribute to columns ======
        seg_row = row("seg_row", 2)            # cos_row dead
        nc.vector.tensor_tensor_scan(seg_row[:], hard_row[:], hard_row[:], 0.0,
                                     op0=ALU.add, op1=ALU.bypass)
        nc.vector.tensor_tensor(seg_row[:], seg_row[:], hard_row[:], op=ALU.subtract)
        nc.vector.memset(seg_row[:, L:LT], -1.0)
        dbg_dump("seg_row", seg_row[:])

        seg_cols = P.tile([128, NLT], dt.float32, name="seg_cols", tag="seg_cols")
        with tc.tile_pool(name="ps_segc", bufs=1, space="PSUM") as PSC:
            pcol = PSC.tile([128, NLT], dt.float32, name="pcol", tag="pcol")
            for f in range(NLT):
                nc.tensor.matmul(pcol[:, f:f + 1], seg_row[0:1, f * 128:(f + 1) * 128],
                                 ones_col[0:1, 0:1], start=True, stop=True)
            nc.vector.tensor_copy(seg_cols[:], pcol[:])

        # ============ segment pooling: f-major, 6 live accx groups ==========
        pooledX = big("pooledX", "B", cols=NSC * 512)   # raw sums (gT slot dead)
        denT = P.tile([8, SHP], dt.float32, name="denT", tag="denT")
        MS = ctx.enter_context(tc.tile_pool(name="mscr", bufs=3))
        with tc.tile_pool(name="ps_seg", bufs=1, space="PSUM") as PS:
            accx = [PS.tile([128, 512], dt.float32, name=f"accx{sc}", tag=f"accx{sc}")
                    for sc in range(NSC)]
            accd = [PS.tile([8, 512], dt.float32, name=f"accd{i}", tag=f"accd{i}")
                    for i in range(2)]
            for f in range(NLT):
                m_f = MS.tile([128, SHP], dt.float32r, name="m_f", tag="m_f")
                nc.vector.tensor_scalar(m_f[:], iota_b[:],
                                        seg_cols[:, f:f + 1], None, op0=ALU.is_equal)
                st, sp = (f == 0), (f == NLT - 1)
                for sc in range(NSC):
                    nc.tensor.matmul(accx[sc][:], m_f[:, sc * 128:(sc + 1) * 128],
                                     fc(vals, f, 0, 512, w=512), start=st, stop=sp)
                ef = e_t[:, f * NH:(f + 1) * NH]
                nc.tensor.matmul(accd[0][:, 0:384], ef, m_f[:, 0:384],
                                 start=st, stop=sp)
                nc.tensor.matmul(accd[1][:, 0:384], ef, m_f[:, 384:SHP],
                                 start=st, stop=sp)
            # denominators to SBUF (transposed layout [head, seg])
            nc.vector.tensor_copy(denT[:, 0:384], accd[0][:, 0:384])
            nc.vector.tensor_copy(denT[:, 384:SHP], accd[1][:, 0:384])
            # raw numerators to SBUF
            for sc in range(NSC):
                nc.scalar.copy(pooledX[:, sc * 512:(sc + 1) * 512], accx[sc][:])

        # rinv = mask / (denom + (1-mask)) in [seg, head] column layout
        pooled = big("pooled", "E", cols=NSC * 512)     # vals slot (dead after accx)
        pooledT = big("pooledT", "A", cols=KC * SHP, tdt=dt.float32r)  # hT dead
        rinvc = P.tile([128, NSC * NH], dt.float32, name="rinvc", tag="rinvc")
        with tc.tile_pool(name="ps_fin", bufs=1, space="PSUM") as PS:
            dtr = PS.tile([128, NSC * NH], dt.float32, name="dtr", tag="dtr")
            for sc in range(NSC):
                nc.tensor.transpose(dtr[:, sc * NH:(sc + 1) * NH],
                                    denT[:, sc * 128:(sc + 1) * 128],
                                    eye[0:NH, 0:NH])
            msk = P.tile([128, NSC * NH], dt.float32, name="msk", tag="msk")
            nc.vector.tensor_scalar(msk[:], dtr[:], 0.0, None, op0=ALU.is_gt)
            nc.vector.tensor_scalar(rinvc[:], msk[:], -1.0, 1.0,
                                    op0=ALU.mult, op1=ALU.add)      # 1-mask
            nc.vector.tensor_tensor(rinvc[:], rinvc[:], dtr[:], op=ALU.add)
            nc.vector.reciprocal(rinvc[:], rinvc[:])
            nc.vector.tensor_tensor(rinvc[:], rinvc[:], msk[:], op=ALU.mult)

            # per-chunk: rinv mult -> 4 transposes -> batched evac
            for sc in range(NSC):
                nc.vector.tensor_tensor(
                    pooled[:, sc * 512:(sc + 1) * 512].rearrange("p (h j) -> p h j", h=NH),
                    pooledX[:, sc * 512:(sc + 1) * 512].rearrange("p (h j) -> p h j", h=NH),
                    rinvc[:, sc * NH:(sc + 1) * NH].unsqueeze(2).broadcast_to([128, NH, HD]),
                    op=ALU.mult)
                ptr = PS.tile([128, 512], dt.float32, name="ptr", tag="ptr", bufs=2)
                for ch in range(KC):
                    nc.tensor.transpose(
                        ptr[:, ch * 128:(ch + 1) * 128],
                        pooled[:, sc * 512 + ch * 128:sc * 512 + (ch + 1) * 128],
                        eye[:])
                nc.vector.tensor_copy(
                    pooledT[:].rearrange("p (c s) -> p c s", c=KC)[:, :, sc * 128:(sc + 1) * 128],
                    ptr[:].rearrange("p (c l) -> p c l", c=KC))
            for sc in range(NSC):
                nrows = min(128, SH - sc * 128)
                if nrows <= 0:
                    break
                acco = PS.tile([128, D], dt.float32, name="acco", tag="acco", bufs=3)
                for ch in range(KC):
                    nc.tensor.matmul(
                        acco[:], pooledT[:, ch * SHP + sc * 128:ch * SHP + (sc + 1) * 128],
                        wpo[:, ch * D:(ch + 1) * D],
                        start=(ch == 0), stop=(ch == KC - 1))
                o_sb = pooled[:, sc * 512:sc * 512 + D].bitcast(dt.float32)
                nc.vector.tensor_copy(o_sb, acco[:])
                nc.sync.dma_start(d_out[sc * 128:sc * 128 + nrows, :], o_sb[0:nrows, :])

    nc.compile()
    return nc


def _rnd11(x):
    """Round fp32 to 11 explicit mantissa bits (fp32r operand precision)."""
    u = np.ascontiguousarray(x).astype(np.float32).view(np.uint32)
    out = ((u + np.uint32(1 << 11)) & np.uint32(0xFFFFF000)).view(np.float32)
    return out


def _sbufify(w):
    """(KC*128, N) row-chunked matrix -> SBUF tile layout [128, KC*N]."""
    n = w.shape[1]
    return np.ascontiguousarray(
        w.reshape(KC, 128, n).transpose(1, 0, 2).reshape(128, KC * n))


def _prep_host(inputs):
    """Host-side prep: transposes, veff fold, hi/lo weight splits, per-core in_maps."""
    f32 = np.float32
    hidden = np.asarray(inputs["hidden"], f32)
    u_noise = np.asarray(inputs["u_noise"], f32)
    W1 = np.asarray(inputs["W1"], f32)
    W2 = np.asarray(inputs["W2"], f32)
    Wq = np.asarray(inputs["Wq"], f32)
    Wk = np.asarray(inputs["Wk"], f32)
    Wpk = np.asarray(inputs["Wpk"], f32)
    Wpv = np.asarray(inputs["Wpv"], f32)
    Wpo = np.asarray(inputs["Wpo"], f32)
    lq = np.asarray(inputs["learned_query"], f32)
    ln_g = np.asarray(inputs["ln_g"], f32)
    ln_b = np.asarray(inputs["ln_b"], f32)
    b1 = np.asarray(inputs["b1"], f32)
    b2 = np.asarray(inputs["b2"], f32)
    lengths = np.asarray(inputs["lengths"], f32)
    bias_f = float(np.asarray(inputs["sim_bias"], f32))
    assert np.all(lengths == 1.0), "kernel specialized for lengths == 1"
    assert np.all(ln_b == 0.0), "kernel assumes ln_b == 0 (fold not implemented)"

    Wpv_f = Wpv * ln_g[None, :]
    Wpk_f = Wpk * ln_g[None, :]
    qh = lq.reshape(NH, HD)
    veff = np.einsum("hj,hji->hi", qh, Wpk_f.reshape(NH, HD, D)) * f32(HD ** -0.5)
    veffT = np.ascontiguousarray(veff.T)
    WpvT = np.ascontiguousarray(Wpv_f.T)

    common = {
        "WpvT": _sbufify(WpvT), "WpoT": _sbufify(np.ascontiguousarray(Wpo.T)),
        "veffT": _sbufify(veffT), "eye": np.eye(128, dtype=f32),
        "bias2": _sbufify(np.ascontiguousarray(np.stack([b1, b2], 1))),
        "wbar_v": WpvT.sum(0, dtype=np.float64).astype(f32).reshape(1, D),
        "wbar_e": veffT.sum(0, dtype=np.float64).astype(f32).reshape(1, NH),
    }
    G = (Wq.T.astype(np.float64) @ Wk.astype(np.float64)).astype(f32)
    for nm, w in (("W1T", W1), ("W2T", W2), ("GT", G.T)):
        wt = np.ascontiguousarray(w.T)
        hi = _rnd11(wt)
        common[nm + "h"] = _sbufify(hi)
        common[nm + "l"] = _sbufify(np.ascontiguousarray(wt - hi))

    # per-batch token stats on host (pure input preprocessing)
    ssq = np.einsum("bld,bld->bl", hidden, hidden, dtype=np.float64)
    rn = (1.0 / np.maximum(np.sqrt(ssq), EPS)).astype(f32)
    mu = hidden.mean(-1, dtype=np.float64).astype(f32)
    var = (ssq / D - mu.astype(np.float64) ** 2)
    rstd = (1.0 / np.sqrt(var + 1e-5)).astype(f32)
    thr = (1.0 - np.clip(u_noise, PEPS, 1.0 - PEPS)).astype(f32)

    in_maps = []
    for c in range(8):
        b, sh = divmod(c, 2)
        m = dict(common)
        hpad = np.zeros((D, LT), f32)
        hpad[:, :L] = hidden[b].T
        m["hT_dev"] = _sbufify(hpad)
        m["thr"] = np.ascontiguousarray(thr[b].reshape(1, L))
        rn_r = np.zeros((1, LT), f32); rn_r[0, :L] = rn[b]
        m["rn_row"] = rn_r
        nm_r = np.zeros((1, LT), f32); nm_r[0, :L] = -mu[b]
        m["negmu_row"] = nm_r
        rs = np.zeros(128 * NLT, f32)
        rs[:L] = rstd[b]
        m["rstd_cols"] = np.ascontiguousarray(rs.reshape(NLT, 128).T)
        m["iota_s"] = (2.0 * np.arange(SHP, dtype=f32) + sh).reshape(1, SHP)
        in_maps.append(m)
    return in_maps, bias_f


def get_nc(bias_f, debug=False):
    key = (round(bias_f, 9), debug)
    if key not in _nc_cache:
        _nc_cache[key] = _build(bias_f, debug=debug)
    return _nc_cache[key]


def kernel(**inputs):
    from concourse.bass_utils import run_bass_kernel_spmd
    in_maps, bias_f = _prep_host(inputs)
    nc = get_nc(bias_f)
    res = run_bass_kernel_spmd(nc, in_maps, list(range(8))).results
    out = np.zeros((B, L, D), np.float32)
    for c in range(8):
        b, sh = divmod(c, 2)
        out[b, sh:sh + 2 * SH:2, :] = res[c]["out_half"]
    return out


# revision 19
# speedup vs baseline: 1.1984x; 1.0196x over previous
"""Trainium2 Bass kernel for nn_BoundaryPredictor2 (B=4, L=1500, D=512, NH=8).

Sharding: 8 cores = batch (4) x segment-half (2). Each core runs the full
boundary chain for its batch (duplicated within the pair) and pools its half
of the segments.

Key algebra vs the reference:
- hard = (soft > 0.5) == (p > 1-u) exactly (logit monotonicity), so the
  boundary decision needs no transcendentals; thr = 1-u precomputed on host.
- mlp(nrm(h)) is shared between the q (tokens :-1) and k (tokens 1:) branches.
- y = nrm(m + z) is never normalized: cos[l] = (qr[l]*kr[l+1])*rny[l]*rny[l+1].
- base[l,h] = hn[l]*veff[h]*HD^-0.5 with veff[h] = qh[h] @ Wpk[64h:64h+64,:],
  so keys are never materialized.
- hn itself is never materialized: x@(W*ln_g) for x=(h-mu)*rstd is computed as
  rstd*(h@Wg - mu*colsum(Wg)); the mu term is a rank-1 outer product matmul
  accumulated into the same PSUM group, rstd folds into a batched base scale
  (for e) and a per-token e2 scale (for vals).
- GEMM precision: margins require |dcos| < 2.3e-4. fp32r rounds operands to
  11 mantissa bits; host splits W = Wh + Wl (round-to-nearest), kernel does
  2 matmul passes Wh@x + Wl@x with x fed as fp32r directly (HW rounds x).
  Residual error ~ 4.7e-5 in cos (simulated, >20x margin headroom).
- Segments are contiguous; pooling = (M^T @ (vals*e)) / (M^T @ e) with M the
  one-hot token->segment matrix built from a prefix scan of hard; f-major
  with 6 live accx groups + 2 transposed-denominator groups so the PE streams.
- All device tiles are DMA'd as single fat 2D copies from host-prearranged
  buffers (SBUF layout precomputed in numpy) to minimize issue count and
  maximize DMA run length.
"""
import numpy as np
from contextlib import ExitStack

import concourse.bass as bass
import concourse.bacc as bacc
import concourse.mybir as mybir
from concourse import tile

dt = mybir.dt
AF = mybir.ActivationFunctionType
ALU = mybir.AluOpType

B, L, D, NH, HD = 4, 1500, 512, 8, 64
EPS = 1e-8
PEPS = 1.1920929e-07
LT = 1536            # padded token count (12 tiles of 128)
NLT = LT // 128      # 12 l-tiles
SH = 750             # segments per core (half of L)
SHP = 768            # padded (6 chunks of 128)
NSC = SHP // 128     # 6 s-chunks
KC = D // 128        # 4 contraction chunks
EXP_SHIFT = -4.0     # constant softmax shift (base observed in [-5.3, 5.6])

_nc_cache = {}


def _build(bias_f, debug=False):
    """Build the SPMD Bass program (same code for all cores; data differs)."""
    nc = bacc.Bacc("TRN2", target_bir_lowering=False, debug=False)

    def din(name, shape, dtype=dt.float32):
        return nc.dram_tensor(name, shape, dtype, kind="ExternalInput").ap()

    # all inputs are host-prearranged to the exact SBUF tile layout
    d_hT = din("hT_dev", (128, KC * LT), dt.float32r)
    d_w = {n + s: din(n + s, (128, KC * D), dt.float32r)
           for n in ("W1T", "W2T", "GT") for s in ("h", "l")}
    d_w["WpvT"] = din("WpvT", (128, KC * D), dt.float32r)
    d_w["WpoT"] = din("WpoT", (128, KC * D), dt.float32r)
    d_veff = din("veffT", (128, KC * NH), dt.float32r)
    d_bias2 = din("bias2", (128, KC * 2))
    d_rn = din("rn_row", (1, LT))
    d_negmu = din("negmu_row", (1, LT), dt.float32r)
    d_rstdc = din("rstd_cols", (128, NLT))
    d_wbar_v = din("wbar_v", (1, D), dt.float32r)
    d_wbar_e = din("wbar_e", (1, NH), dt.float32r)
    d_iota = din("iota_s", (1, SHP))
    d_eye = din("eye", (128, 128))
    d_thr = din("thr", (1, L))
    d_out = nc.dram_tensor("out_half", (SH, D), dt.float32, kind="ExternalOutput").ap()
    dbg = {}
    if debug:
        for nm in ("cos_row", "hard_row", "seg_row", "rny_row"):
            dbg[nm] = nc.dram_tensor(nm, (1, LT), dt.float32, kind="ExternalOutput").ap()

        def dbg_dump(nm, ap):
            nc.sync.dma_start(dbg[nm][:], ap)
    else:
        def dbg_dump(nm, ap):
            pass

    with tile.TileContext(nc) as tc, ExitStack() as ctx:
        P = ctx.enter_context(tc.tile_pool(name="main", bufs=1))

        def fc(t, k, lo, n, w=LT):
            return t[:, k * w + lo:k * w + lo + n]

        def row(role, tag):
            return P.tile([1, LT], dt.float32, name=role, tag=f"row{tag}")

        def big(name, tag, cols=KC * LT, tdt=dt.float32):
            return P.tile([128, cols], tdt, name=name, tag=tag)

        # ---------- wave-1 DMAs spread across the three DGE queues ----------
        # sync: hidden^T per k-chunk (zT[k] unblocks early), later G/Wpo + outs
        hT = P.tile([128, KC * LT], dt.float32r, name="hT", tag="A")
        for k in range(KC):
            nc.sync.dma_start(fc(hT, k, 0, LT), d_hT[:, k * LT:(k + 1) * LT])

        def hf(k, lo, n):      # fp32 view of hT chunk
            return fc(hT, k, lo, n).bitcast(dt.float32)

        # scalar: W1 pair + biases (the MLP1 critical path)
        def wtile(slot):
            return P.tile([128, KC * D], dt.float32r, name=slot, tag=slot)

        w1h, w1l = wtile("s0"), wtile("s1")
        w2h, w2l = wtile("s2"), wtile("s3")
        wpv = wtile("s4")
        nc.scalar.dma_start(w1h[:], d_w["W1Th"][:])
        nc.scalar.dma_start(w1l[:], d_w["W1Tl"][:])
        bias2 = P.tile([128, KC * 2], dt.float32, name="bias2_sb", tag="bias2_sb")
        nc.scalar.dma_start(bias2[:], d_bias2[:])

        def b1c(do):
            return bias2[:, do * 2:do * 2 + 1]

        def b2c(do):
            return bias2[:, do * 2 + 1:do * 2 + 2]

        # gpsimd: rn first (feeds bc1), W2 pair, Wpv/veff, then small consts
        rn_row = row("rn_row", 4)
        nc.gpsimd.dma_start(rn_row[:], d_rn[:])
        bc1 = P.tile([128, LT], dt.float32, name="bc1", tag="bc1")
        nc.gpsimd.partition_broadcast(bc1[:], rn_row[:])
        nc.gpsimd.dma_start(w2h[:], d_w["W2Th"][:])
        nc.gpsimd.dma_start(w2l[:], d_w["W2Tl"][:])
        nc.gpsimd.dma_start(wpv[:], d_w["WpvT"][:])
        veff = P.tile([128, KC * NH], dt.float32r, name="veff_sb", tag="veff_sb")
        nc.gpsimd.dma_start(veff[:], d_veff[:])
        eye = P.tile([128, 128], dt.float32, name="eye_sb", tag="eye_sb")
        nc.gpsimd.dma_start(eye[:], d_eye[:])
        iota_b = P.tile([128, SHP], dt.float32, name="iota_b", tag="iota_b")
        nc.gpsimd.dma_start(iota_b[:], d_iota[:].partition_broadcast(128))
        rstdc = P.tile([128, NLT], dt.float32, name="rstdc", tag="rstdc")
        nc.gpsimd.dma_start(rstdc[:], d_rstdc[:])
        wbar_v = P.tile([1, D], dt.float32r, name="wbar_v", tag="wbar_v")
        nc.gpsimd.dma_start(wbar_v[:], d_wbar_v[:])
        wbar_e = P.tile([1, NH], dt.float32r, name="wbar_e", tag="wbar_e")
        nc.gpsimd.dma_start(wbar_e[:], d_wbar_e[:])
        negmu = P.tile([1, LT], dt.float32r, name="negmu", tag="row6")
        nc.gpsimd.dma_start(negmu[:], d_negmu[:])
        thr_row = row("thr_row", 3)
        nc.gpsimd.dma_start(thr_row[:, 0:L], d_thr[:])

        ones_col = P.tile([128, 1], dt.float32, name="ones_col", tag="ones_col")
        nc.vector.memset(ones_col[:], 1.0)
        ones_r = P.tile([128, 1], dt.float32r, name="ones_r", tag="ones_r")
        nc.scalar.copy(ones_r[:], ones_col[:])
        eshift = P.tile([128, 1], dt.float32, name="eshift", tag="eshift")
        nc.vector.memset(eshift[:], EXP_SHIFT)

        # ============ z^T = h^T * rn ============
        zT = big("zT", "C", tdt=dt.float32r)
        for k in range(KC):
            nc.vector.tensor_tensor(fc(zT, k, 0, LT), hf(k, 0, LT),
                                    bc1[:], op=ALU.mult)

        # ============ 2-pass fp32r GEMM: out = (Wh + Wl) @ round11(x) ========
        def w_matmul(wh, wl, rhs, evac):
            with tc.tile_pool(name="ps_mm", bufs=4, space="PSUM") as PS:
                for lc in range(LT // 512):
                    for do in range(KC):
                        acc = PS.tile([128, 512], dt.float32, name="mmacc", tag="mmacc")
                        i = 0
                        for k in range(KC):
                            x_ap = fc(rhs, k, lc * 512, 512)
                            whk = wh[:, k * D + do * 128:k * D + (do + 1) * 128]
                            wlk = wl[:, k * D + do * 128:k * D + (do + 1) * 128]
                            nc.tensor.matmul(acc[:], whk, x_ap,
                                             start=(i == 0), stop=False)
                            i += 1
                            nc.tensor.matmul(acc[:], wlk, x_ap,
                                             start=False, stop=(i == 2 * KC - 1))
                            i += 1
                        evac(acc, do, lc)

        gT = big("gT", "B", tdt=dt.float32r)

        def evac_gelu(acc, do, lc):
            nc.scalar.activation(fc(gT, do, lc * 512, 512), acc[:], AF.Gelu,
                                 bias=b1c(do))

        w_matmul(w1h, w1l, zT, evac_gelu)

        yT = big("yT", "E", tdt=dt.float32r)
        ssy_row = row("ssy_row", 1)
        tmp_row = row("tmp_row", 5)

        # MLP2 with fused y^2 scratch + ssy reduction (pipelines into chain)
        with tc.tile_pool(name="ps_rowy", bufs=2, space="PSUM") as PSY, \
                tc.tile_pool(name="sqpool", bufs=5) as SQ:
            ssy_acc = {}

            def evac_y(acc, do, lc):
                nc.vector.scalar_tensor_tensor(
                    fc(yT, do, lc * 512, 512), acc[:], b2c(do),
                    fc(zT, do, lc * 512, 512).bitcast(dt.float32),
                    op0=ALU.add, op1=ALU.add)
                yv = fc(yT, do, lc * 512, 512).bitcast(dt.float32)
                sq = SQ.tile([128, 512], dt.float32r, name="sq", tag="sq")
                nc.gpsimd.tensor_tensor(sq[:], yv, yv, op=ALU.mult)
                if do == 0:
                    ssy_acc[lc] = PSY.tile([1, 512], dt.float32, name="racy", tag="racy")
                nc.tensor.matmul(ssy_acc[lc][:], ones_r[:], sq[:],
                                 start=(do == 0), stop=(do == KC - 1))
                if do == KC - 1:
                    nc.scalar.copy(ssy_row[:, lc * 512:(lc + 1) * 512], ssy_acc[lc][:])

            w_matmul(w2h, w2l, gT, evac_y)

        # G chain reuses w1 slots; wpo reuses w2h slot (sync queue, ordered).
        gqh, gql = w1h, w1l
        nc.sync.dma_start(gqh[:], d_w["GTh"][:])
        nc.sync.dma_start(gql[:], d_w["GTl"][:])
        wpo = w2h
        nc.sync.dma_start(wpo[:], d_w["WpoT"][:])

        # ============ rny (rows on vector while G chain runs on PE) =========
        rny_row = row("rny_row", 4)            # rn_row dead
        nc.scalar.activation(tmp_row[:], ssy_row[:], AF.Sqrt)
        nc.vector.tensor_scalar_max(tmp_row[:], tmp_row[:], EPS)
        nc.vector.reciprocal(rny_row[:], tmp_row[:])
        dbg_dump("rny_row", rny_row[:])
        rr_row = row("rr_row", 1)              # ssy_row dead; rr[l] = rny[l]*rny[l+1]
        nc.vector.memset(rr_row[:, L - 1:LT], 0.0)
        nc.vector.tensor_tensor(rr_row[:, 0:L - 1], rny_row[:, 0:L - 1],
                                rny_row[:, 1:L], op=ALU.mult)

        # ============ qr, kr, cos ============
        # gq = y @ G with G = Wq.T @ Wk; cos[l] = gq[l] . y[l+1]
        prodT = big("prodT", "C", tdt=dt.float32r)   # reuse zT slot (dead)

        def evac_gq(acc, do, lc):
            # prod[:, l] = gq[:, l] * y[:, l+1]; pad/tail zeroed after
            lo = lc * 512
            n = 512 if lo + 512 < L else (L - 1 - lo)
            nc.vector.tensor_tensor(fc(prodT, do, lo, n), acc[0:128, 0:n],
                                    fc(yT, do, lo + 1, n).bitcast(dt.float32),
                                    op=ALU.mult)
            if n < 512:
                nc.vector.tensor_scalar(fc(prodT, do, lo + n, LT - lo - n),
                                        acc[0:128, 0:LT - lo - n], 0.0, None,
                                        op0=ALU.mult)

        w_matmul(gqh, gql, yT, evac_gq)
        # cos = (ones @ prod) * rr, scaling fused into the psum evacuation
        cos_row = row("cos_row", 2)
        with tc.tile_pool(name="ps_rowc", bufs=2, space="PSUM") as PSR:
            for lc in range(LT // 512):
                acc = PSR.tile([1, 512], dt.float32, name="racc2", tag="racc2")
                for k in range(KC):
                    nc.tensor.matmul(acc[:], ones_r[:], fc(prodT, k, lc * 512, 512),
                                     start=(k == 0), stop=(k == KC - 1))
                nc.vector.tensor_tensor(cos_row[:, lc * 512:(lc + 1) * 512], acc[:],
                                        rr_row[:, lc * 512:(lc + 1) * 512], op=ALU.mult)
        dbg_dump("cos_row", cos_row[:])

        # ============ pooling-side tensors (PE filler during row math) ======
        e_t = P.tile([128, NLT * NH], dt.float32r, name="e_t", tag="e_t")
        e2_t = P.tile([128, NLT * NH], dt.float32, name="e2_t", tag="e2_t")
        base_all = P.tile([128, NLT * NH], dt.float32, name="base_all", tag="base_all")
        vals = big("vals", "E", cols=NLT * 512, tdt=dt.float32r)  # yT dead

        rstd3 = rstdc[:].unsqueeze(2).broadcast_to([128, NLT, NH])
        with tc.tile_pool(name="ps_pv", bufs=1, space="PSUM") as PS:
            bcc = PS.tile([128, NLT * NH], dt.float32, name="bcc", tag="bcc")
            for f in range(NLT):
                nmu = negmu[:, f * 128:(f + 1) * 128]
                bf = bcc[:, f * NH:(f + 1) * NH]
                nc.tensor.matmul(bf, nmu, wbar_e[:], start=True, stop=False)
                for k in range(KC):
                    nc.tensor.matmul(bf, fc(hT, k, f * 128, 128),
                                     veff[:, k * NH:(k + 1) * NH],
                                     start=False, stop=(k == KC - 1))
            # base = rstd * base_raw; e = exp(base + shift); e2 = e * rstd
            nc.vector.tensor_tensor(base_all[:].rearrange("p (f h) -> p f h", f=NLT),
                                    bcc[:].rearrange("p (f h) -> p f h", f=NLT),
                                    rstd3, op=ALU.mult)
            nc.scalar.activation(e_t[:], base_all[:], AF.Exp, bias=eshift[:])
            nc.vector.tensor_tensor(e2_t[:].rearrange("p (f h) -> p f h", f=NLT),
                                    e_t[:].bitcast(dt.float32).rearrange("p (f h) -> p f h", f=NLT),
                                    rstd3, op=ALU.mult)
            for f in range(NLT):
                nmu = negmu[:, f * 128:(f + 1) * 128]
                acc = PS.tile([128, 512], dt.float32, name="vacc", tag="vacc", bufs=4)
                nc.tensor.matmul(acc[:], nmu, wbar_v[:], start=True, stop=False)
                for k in range(KC):
                    nc.tensor.matmul(acc[:], fc(hT, k, f * 128, 128),
                                     wpv[:, k * D:(k + 1) * D],
                                     start=False, stop=(k == KC - 1))
                # X = vals_raw * e2, fused psum evacuation
                nc.vector.tensor_tensor(
                    fc(vals, f, 0, 512, w=512).rearrange("p (h j) -> p h j", h=NH),
                    acc[:].rearrange("p (h j) -> p h j", h=NH),
                    e2_t[:, f * NH:(f + 1) * NH].unsqueeze(2).broadcast_to([128, NH, HD]),
                    op=ALU.mult)

        # ============ boundary decision: hard = (p > thr) ============
        p_row = row("p_row", 1)
        nc.vector.tensor_scalar(p_row[:, 0:L - 1], cos_row[:, 0:L - 1], -0.5,
                                0.5 - 0.5 * bias_f, op0=ALU.mult, op1=ALU.add)
        nc.vector.memset(p_row[:, L - 1:LT], 0.0)
        nc.vector.tensor_scalar(p_row[:, 0:L], p_row[:, 0:L], PEPS, 1.0 - PEPS,
                                op0=ALU.max, op1=ALU.min)
        hard_row = row("hard_row", 4)          # rny_row dead
        nc.vector.memset(hard_row[:], 0.0)
        nc.vector.tensor_tensor(hard_row[:, 0:L], p_row[:, 0:L], thr_row[:, 0:L],
                                op=ALU.is_gt)
        hsum = P.tile([1, 1], dt.float32, name="hsum", tag="hsum")
        nc.vector.tensor_reduce(hsum[:], hard_row[:, 0:L], axis=mybir.AxisListType.X,
                                op=ALU.add)
        nc.vector.tensor_scalar(hsum[:], hsum[:], 0.0, None, op0=ALU.is_equal)
        nc.vector.tensor_tensor(hard_row[:, L - 1:L], hard_row[:, L - 1:L], hsum[:],
                                op=ALU.max)
        dbg_dump("hard_row", hard_row[:])

        # ============ seg = exclusive prefix sum; distribute to columns ======
        seg_row = row("seg_row", 2)            # cos_row dead
        nc.vector.tensor_tensor_scan(seg_row[:], hard_row[:], hard_row[:], 0.0,
                                     op0=ALU.add, op1=ALU.bypass)
        nc.vector.tensor_tensor(seg_row[:], seg_row[:], hard_row[:], op=ALU.subtract)
        nc.vector.memset(seg_row[:, L:LT], -1.0)
        dbg_dump("seg_row", seg_row[:])

        seg_cols = P.tile([128, NLT], dt.float32, name="seg_cols", tag="seg_cols")
        with tc.tile_pool(name="ps_segc", bufs=1, space="PSUM") as PSC:
            pcol = PSC.tile([128, NLT], dt.float32, name="pcol", tag="pcol")
            for f in range(NLT):
                nc.tensor.matmul(pcol[:, f:f + 1], seg_row[0:1, f * 128:(f + 1) * 128],
                                 ones_col[0:1, 0:1], start=True, stop=True)
            nc.vector.tensor_copy(seg_cols[:], pcol[:])

        # ============ segment pooling: f-major, 6 live accx groups ==========
        pooledX = big("pooledX", "B", cols=NSC * 512)   # raw sums (gT slot dead)
        denT = P.tile([8, SHP], dt.float32, name="denT", tag="denT")
        MS = ctx.enter_context(tc.tile_pool(name="mscr", bufs=3))
        with tc.tile_pool(name="ps_seg", bufs=1, space="PSUM") as PS:
            accx = [PS.tile([128, 512], dt.float32, name=f"accx{sc}", tag=f"accx{sc}")
                    for sc in range(NSC)]
            accd = [PS.tile([8, 512], dt.float32, name=f"accd{i}", tag=f"accd{i}")
                    for i in range(2)]
            for f in range(NLT):
                m_f = MS.tile([128, SHP], dt.float32r, name="m_f", tag="m_f")
                nc.vector.tensor_scalar(m_f[:], iota_b[:],
                                        seg_cols[:, f:f + 1], None, op0=ALU.is_equal)
                st, sp = (f == 0), (f == NLT - 1)
                for sc in range(NSC):
                    nc.tensor.matmul(accx[sc][:], m_f[:, sc * 128:(sc + 1) * 128],
                                     fc(vals, f, 0, 512, w=512), start=st, stop=sp)
                ef = e_t[:, f * NH:(f + 1) * NH]
                nc.tensor.matmul(accd[0][:, 0:384], ef, m_f[:, 0:384],
                                 start=st, stop=sp)
                nc.tensor.matmul(accd[1][:, 0:384], ef, m_f[:, 384:SHP],
                                 start=st, stop=sp)
            # denominators to SBUF (transposed layout [head, seg])
            nc.vector.tensor_copy(denT[:, 0:384], accd[0][:, 0:384])
            nc.vector.tensor_copy(denT[:, 384:SHP], accd[1][:, 0:384])
            # raw numerators to SBUF
            for sc in range(NSC):
                nc.scalar.copy(pooledX[:, sc * 512:(sc + 1) * 512], accx[sc][:])

        # rinv = mask / (denom + (1-mask)) in [seg, head] column layout
        pooled = big("pooled", "E", cols=NSC * 512)     # vals slot (dead after accx)
        pooledT = big("pooledT", "A", cols=KC * SHP, tdt=dt.float32r)  # hT dead
        rinvc = P.tile([128, NSC * NH], dt.float32, name="rinvc", tag="rinvc")
        with tc.tile_pool(name="ps_fin", bufs=1, space="PSUM") as PS:
            dtr = PS.tile([128, NSC * NH], dt.float32, name="dtr", tag="dtr")
            for sc in range(NSC):
                nc.tensor.transpose(dtr[:, sc * NH:(sc + 1) * NH],
                                    denT[:, sc * 128:(sc + 1) * 128],
                                    eye[0:NH, 0:NH])
            msk = P.tile([128, NSC * NH], dt.float32, name="msk", tag="msk")
            nc.vector.tensor_scalar(msk[:], dtr[:], 0.0, None, op0=ALU.is_gt)
            nc.vector.tensor_scalar(rinvc[:], msk[:], -1.0, 1.0,
                                    op0=ALU.mult, op1=ALU.add)      # 1-mask
            nc.vector.tensor_tensor(rinvc[:], rinvc[:], dtr[:], op=ALU.add)
            nc.vector.reciprocal(rinvc[:], rinvc[:])
            nc.vector.tensor_tensor(rinvc[:], rinvc[:], msk[:], op=ALU.mult)

            # per-chunk: rinv mult -> 4 transposes -> batched evac
            for sc in range(NSC):
                nc.vector.tensor_tensor(
                    pooled[:, sc * 512:(sc + 1) * 512].rearrange("p (h j) -> p h j", h=NH),
                    pooledX[:, sc * 512:(sc + 1) * 512].rearrange("p (h j) -> p h j", h=NH),
                    rinvc[:, sc * NH:(sc + 1) * NH].unsqueeze(2).broadcast_to([128, NH, HD]),
                    op=ALU.mult)
                ptr = PS.tile([128, 512], dt.float32, name="ptr", tag="ptr", bufs=2)
                for ch in range(KC):
                    nc.tensor.transpose(
                        ptr[:, ch * 128:(ch + 1) * 128],
                        pooled[:, sc * 512 + ch * 128:sc * 512 + (ch + 1) * 128],
                        eye[:])
                nc.vector.tensor_copy(
                    pooledT[:].rearrange("p (c s) -> p c s", c=KC)[:, :, sc * 128:(sc + 1) * 128],
                    ptr[:].rearrange("p (c l) -> p c l", c=KC))
            for sc in range(NSC):
                nrows = min(128, SH - sc * 128)
                if nrows <= 0:
                    break
                acco = PS.tile([128, D], dt.float32, name="acco", tag="acco", bufs=3)
                for ch in range(KC):
                    nc.tensor.matmul(
                        acco[:], pooledT[:, ch * SHP + sc * 128:ch * SHP + (sc + 1) * 128],
                        wpo[:, ch * D:(ch + 1) * D],
                        start=(ch == 0), stop=(ch == KC - 1))
                o_sb = pooled[:, sc * 512:sc * 512 + D].bitcast(dt.float32)
                nc.vector.tensor_copy(o_sb, acco[:])
                nc.sync.dma_start(d_out[sc * 128:sc * 128 + nrows, :], o_sb[0:nrows, :])

    nc.compile()
    return nc


def _rnd11(x):
    """Round fp32 to 11 explicit mantissa bits (fp32r operand precision)."""
    u = np.ascontiguousarray(x).astype(np.float32).view(np.uint32)
    out = ((u + np.uint32(1 << 11)) & np.uint32(0xFFFFF000)).view(np.float32)
    return out


def _sbufify(w):
    """(KC*128, N) row-chunked matrix -> SBUF tile layout [128, KC*N]."""
    n = w.shape[1]
    return np.ascontiguousarray(
        w.reshape(KC, 128, n).transpose(1, 0, 2).reshape(128, KC * n))


def _prep_host(inputs):
    """Host-side prep: transposes, veff fold, hi/lo weight splits, per-core in_maps."""
    f32 = np.float32
    hidden = np.asarray(inputs["hidden"], f32)
    u_noise = np.asarray(inputs["u_noise"], f32)
    W1 = np.asarray(inputs["W1"], f32)
    W2 = np.asarray(inputs["W2"], f32)
    Wq = np.asarray(inputs["Wq"], f32)
    Wk = np.asarray(inputs["Wk"], f32)
    Wpk = np.asarray(inputs["Wpk"], f32)
    Wpv = np.asarray(inputs["Wpv"], f32)
    Wpo = np.asarray(inputs["Wpo"], f32)
    lq = np.asarray(inputs["learned_query"], f32)
    ln_g = np.asarray(inputs["ln_g"], f32)
    ln_b = np.asarray(inputs["ln_b"], f32)
    b1 = np.asarray(inputs["b1"], f32)
    b2 = np.asarray(inputs["b2"], f32)
    lengths = np.asarray(inputs["lengths"], f32)
    bias_f = float(np.asarray(inputs["sim_bias"], f32))
    assert np.all(lengths == 1.0), "kernel specialized for lengths == 1"
    assert np.all(ln_b == 0.0), "kernel assumes ln_b == 0 (fold not implemented)"

    Wpv_f = Wpv * ln_g[None, :]
    Wpk_f = Wpk * ln_g[None, :]
    qh = lq.reshape(NH, HD)
    veff = np.einsum("hj,hji->hi", qh, Wpk_f.reshape(NH, HD, D)) * f32(HD ** -0.5)
    veffT = np.ascontiguousarray(veff.T)
    WpvT = np.ascontiguousarray(Wpv_f.T)

    common = {
        "WpvT": _sbufify(WpvT), "WpoT": _sbufify(np.ascontiguousarray(Wpo.T)),
        "veffT": _sbufify(veffT), "eye": np.eye(128, dtype=f32),
        "bias2": _sbufify(np.ascontiguousarray(np.stack([b1, b2], 1))),
        "wbar_v": WpvT.sum(0, dtype=np.float64).astype(f32).reshape(1, D),
        "wbar_e": veffT.sum(0, dtype=np.float64).astype(f32).reshape(1, NH),
    }
    G = (Wq.T.astype(np.float64) @ Wk.astype(np.float64)).astype(f32)
    for nm, w in (("W1T", W1), ("W2T", W2), ("GT", G.T)):
        wt = np.ascontiguousarray(w.T)
        hi = _rnd11(wt)
        common[nm + "h"] = _sbufify(hi)
        common[nm + "l"] = _sbufify(np.ascontiguousarray(wt - hi))

    # per-batch token stats on host (pure input preprocessing)
    ssq = np.einsum("bld,bld->bl", hidden, hidden, dtype=np.float64)
    rn = (1.0 / np.maximum(np.sqrt(ssq), EPS)).astype(f32)
    mu = hidden.mean(-1, dtype=np.float64).astype(f32)
    var = (ssq / D - mu.astype(np.float64) ** 2)
    rstd = (1.0 / np.sqrt(var + 1e-5)).astype(f32)
    thr = (1.0 - np.clip(u_noise, PEPS, 1.0 - PEPS)).astype(f32)

    in_maps = []
    for c in range(8):
        b, sh = divmod(c, 2)
        m = dict(common)
        hpad = np.zeros((D, LT), f32)
        hpad[:, :L] = hidden[b].T
        m["hT_dev"] = _sbufify(hpad)
        m["thr"] = np.ascontiguousarray(thr[b].reshape(1, L))
        rn_r = np.zeros((1, LT), f32); rn_r[0, :L] = rn[b]
        m["rn_row"] = rn_r
        nm_r = np.zeros((1, LT), f32); nm_r[0, :L] = -mu[b]
        m["negmu_row"] = nm_r
        rs = np.zeros(128 * NLT, f32)
        rs[:L] = rstd[b]
        m["rstd_cols"] = np.ascontiguousarray(rs.reshape(NLT, 128).T)
        m["iota_s"] = (2.0 * np.arange(SHP, dtype=f32) + sh).reshape(1, SHP)
        in_maps.append(m)
    return in_maps, bias_f


def get_nc(bias_f, debug=False):
    key = (round(bias_f, 9), debug)
    if key not in _nc_cache:
        _nc_cache[key] = _build(bias_f, debug=debug)
    return _nc_cache[key]


def kernel(**inputs):
    from concourse.bass_utils import run_bass_kernel_spmd
    in_maps, bias_f = _prep_host(inputs)
    nc = get_nc(bias_f)
    res = run_bass_kernel_spmd(nc, in_maps, list(range(8))).results
    out = np.zeros((B, L, D), np.float32)
    for c in range(8):
        b, sh = divmod(c, 2)
        out[b, sh:sh + 2 * SH:2, :] = res[c]["out_half"]
    return out
